# revision 16
# baseline (speedup 1.0000x reference)
"""DiffOfGaussians Trainium2 kernel (v10: contiguous pipeline).

Math:
  out[b,u] = sum_{h,w,c} inputs[b,h,w,c] * F[h,w,u] + bias[u]
  F[h,w,u] = sum_p sgn_p * coef_p[u] * gx_p[w,u] * gy_p[h,u]   (separable)

Sharding: H split across 8 cores (16 rows each).  Host packs the shard as
bf16 [hp8, w128, (c16, h2, b64)]: w on SBUF partitions, c outermost so the
c-reduce is a unit-stride bf16 add tree, (h,b) innermost-contiguous so
every downstream op uses >=64-element unit-stride runs.  The two HWDGE
queues stream all 8 chunks up front into one big SBUF tile.  Chunk pairs
(h4) flow: L1+L2 (DVE) -> L3+L4 (GpSimd) -> 4 matmuls f=256 (PE)
-> 2 DVE mults by the gy table straight from PSUM -> h4 tree + path sum
(GpSimd) -> 2-level fp32 accumulate tree (DVE).  One [128,(k2,b64)] fp32
DMA out; the host sums 8 per-core partials and adds the bias.
"""

import sys

for _p in ("/opt/trn_rl_repo",):
    if _p not in sys.path:
        sys.path.insert(0, _p)

import numpy as np

import concourse.bass as bass
import concourse.tile as tile
from concourse import bacc, masks, mybir
from concourse.bass_utils import run_bass_kernel_spmd

F32 = mybir.dt.float32
BF16 = mybir.dt.bfloat16
I32 = mybir.dt.int32
OP = mybir.AluOpType
AF = mybir.ActivationFunctionType

B, H, W, C, U = 64, 128, 128, 16, 256
NCORES = 8
HSH = H // NCORES      # 16 h rows per core
NCH = HSH // 2         # 8 chunks of 2 h rows
NPAIR = NCH // 2       # 4 pairs of chunks (h4 each)
INV2PI = float(1.0 / (2.0 * np.pi))

_CACHE = {}


def _ap(t, off, dims):
    """AP over tile t at element offset off with free dims [(stride, n), ...]."""
    base = t[:]
    return bass.AP(
        tensor=base.tensor, offset=base.offset + off,
        ap=[base.ap[0]] + [list(d) for d in dims],
    )


def _build_kernel():
    nc = bacc.Bacc(
        "TRN2",
        target_bir_lowering=False,
        debug=False,
        num_devices=NCORES,
    )

    # input chunks: x[hp, w, (c16, h2, b64)]; h = 16*core + 2*hp + h2
    x_d = nc.dram_tensor("x", [NCH, 128, 2048], BF16, kind="ExternalInput").ap()
    # yc broadcast row (per-core h values)
    aux_d = nc.dram_tensor("aux", [1, 16], F32, kind="ExternalInput").ap()
    # per-partition params (a1k0,a1k1,a2k0,a2k1,s1k0,s1k1,s2k0,s2k1,uyk0,uyk1,uxk0,uxk1)
    prm_d = nc.dram_tensor("prm", [128, 12], F32, kind="ExternalInput").ap()
    # out[u_lo, (k2, b64)] fp32 partial
    out_d = nc.dram_tensor("out", [128, 128], F32, kind="ExternalOutput").ap()

    with tile.TileContext(nc) as tc:
        with (
            tc.tile_pool(name="singles", bufs=1) as sg,
            tc.tile_pool(name="ta", bufs=2) as ta_pool,
            tc.tile_pool(name="tb", bufs=2) as tb_pool,
            tc.tile_pool(name="tcp", bufs=2) as tc_pool,
            tc.tile_pool(name="ep", bufs=2) as ep_pool,
            tc.tile_pool(name="ptr", bufs=2, space="PSUM") as tr_psum,
            tc.tile_pool(name="pz", bufs=6, space="PSUM") as pz_pool,
        ):
            # ---- DMAs first: params on scalar, chunks on both HW queues ----
            pp = sg.tile([128, 14], F32)
            nc.scalar.dma_start(out=pp[:, 0:12], in_=prm_d)
            yc_r = sg.tile([128, 16], F32)
            aux_bc = bass.AP(
                tensor=aux_d.tensor, offset=aux_d.offset, ap=[[0, 128], [1, 16]]
            )
            nc.scalar.dma_start(out=yc_r[:], in_=aux_bc)

            X = sg.tile([128, NCH * 2048], BF16)
            for hp in range(NCH):
                eng = nc.sync if hp % 2 == 0 else nc.scalar
                eng.dma_start(out=X[:, hp * 2048 : (hp + 1) * 2048], in_=x_d[hp])

            # ---- constants ----
            identity = sg.tile([128, 128], BF16)
            masks.make_identity(nc, identity[:])
            iota_i = sg.tile([128, 128], I32)
            nc.gpsimd.iota(iota_i[:], pattern=[[1, 128]], base=0, channel_multiplier=0)
            iota_f = sg.tile([128, 128], F32)
            nc.gpsimd.tensor_copy(iota_f[:], iota_i[:])

            # ---- per-partition (u_lo) param math ----
            nc.gpsimd.tensor_add(pp[:, 12:14], pp[:, 4:6], pp[:, 6:8])
            rct = sg.tile([128, 4], F32)  # 1/sigma, cols (p,k)
            nc.vector.reciprocal(rct[:, 0:2], pp[:, 4:6])
            nc.vector.reciprocal(rct[:, 2:4], pp[:, 12:14])
            nis4 = sg.tile([128, 4], F32)
            nc.gpsimd.tensor_scalar_mul(nis4[:], rct[:], -0.5)
            coef4 = sg.tile([128, 4], F32)
            nc.gpsimd.tensor_tensor(coef4[:], pp[:, 0:4], rct[:], op=OP.mult)
            nc.gpsimd.tensor_scalar_mul(coef4[:, 0:2], coef4[:, 0:2], INV2PI)
            nc.gpsimd.tensor_scalar_mul(coef4[:, 2:4], coef4[:, 2:4], -INV2PI)
            nuy2 = sg.tile([128, 2], F32)
            nc.gpsimd.tensor_scalar_mul(nuy2[:], pp[:, 8:10], -1.0)
            nux2 = sg.tile([128, 2], F32)
            nc.gpsimd.tensor_scalar_mul(nux2[:], pp[:, 10:12], -1.0)

            gx = []
            for p in range(2):
                g = sg.tile([128, 256], BF16, tag=f"gx{p}")
                gx.append(g)
            with nc.allow_low_precision("bf16 tables; 2e-2 rel-err budget"):
                # ---- gx tables: [u_lo, w] -> PE transpose -> [w, u] ----
                for k in range(2):
                    d2 = sg.tile([128, 128], F32, tag=f"d2_{k}")
                    nc.scalar.activation(
                        d2[:], iota_f[:], AF.Square, bias=nux2[:, k : k + 1]
                    )
                    for p in range(2):
                        e = ta_pool.tile([128, 128], BF16, tag="gxe")
                        nc.scalar.activation(
                            e[:], d2[:], AF.Exp, bias=0.0,
                            scale=nis4[:, 2 * p + k : 2 * p + k + 1],
                        )
                        ps = tr_psum.tile([128, 128], BF16, tag="trp")
                        nc.tensor.transpose(ps[:], e[:], identity[:])
                        nc.scalar.activation(
                            gx[p][:, k * 128 : (k + 1) * 128], ps[:], AF.Copy
                        )

                # ---- gy table [u_lo, (k2, p2, h16)] bf16, sign+coef folded ----
                gy_sb = sg.tile([128, 64], BF16)
                for k in range(2):
                    dy2 = sg.tile([128, 16], F32, tag=f"dy2_{k}")
                    nc.scalar.activation(
                        dy2[:], yc_r[:], AF.Square, bias=nuy2[:, k : k + 1]
                    )
                    for p in range(2):
                        eg = sg.tile([128, 16], F32, tag=f"eg{p}{k}")
                        nc.scalar.activation(
                            eg[:], dy2[:], AF.Exp, bias=0.0,
                            scale=nis4[:, 2 * p + k : 2 * p + k + 1],
                        )
                        nc.gpsimd.tensor_scalar_mul(
                            gy_sb[:, k * 32 + p * 16 : k * 32 + p * 16 + 16],
                            eg[:], coef4[:, 2 * p + k : 2 * p + k + 1],
                        )

                # xr[w, (h16, b64)]
                xr = sg.tile([128, 1024], BF16)
                ctbs = []

                # ---- per-pair pipeline ----
                for q in range(NPAIR):
                    # L1 (DVE): chunks {2q,2q+1} c16->c8 into (c8, chunk2, hb128)
                    a = ta_pool.tile([128, 2048], BF16, tag="a")
                    s0 = _ap(X, q * 4096, [(2048, 2), (128, 8), (1, 128)])
                    s1 = _ap(X, q * 4096 + 1024, [(2048, 2), (128, 8), (1, 128)])
                    d = _ap(a, 0, [(128, 2), (256, 8), (1, 128)])
                    nc.vector.tensor_tensor(d, s0, s1, op=OP.add)
                    # L2 (DVE): c8 -> c4, contiguous halves
                    b_ = tb_pool.tile([128, 1024], BF16, tag="b")
                    nc.vector.tensor_add(b_[:], a[:, 0:1024], a[:, 1024:2048])
                    # L3 (gp): c4 -> c2
                    c_ = tc_pool.tile([128, 512], BF16, tag="c")
                    nc.gpsimd.tensor_add(c_[:], b_[:, 0:512], b_[:, 512:1024])
                    # L4 (gp): c2 -> 1, straight into xr[:, q*256:(q+1)*256]
                    nc.gpsimd.tensor_add(
                        xr[:, q * 256 : (q + 1) * 256], c_[:, 0:256], c_[:, 256:512]
                    )

                    # 4 matmuls f=256: psum z_k[u, (p2, h4, b64)]
                    rhs = xr[:, q * 256 : (q + 1) * 256]
                    zql = []
                    for k in range(2):
                        z = pz_pool.tile([128, 512], F32, tag="z")
                        for p in range(2):
                            nc.tensor.matmul(
                                z[:, p * 256 : (p + 1) * 256],
                                gx[p][:, k * 128 : (k + 1) * 128],
                                rhs,
                                start=True,
                                stop=True,
                            )
                        zql.append(z)

                    # 2 DVE mults by gy straight from PSUM -> e0 bf16
                    e0 = ep_pool.tile([128, 1024], BF16, tag="e0")
                    for k in range(2):
                        zin = _ap(zql[k], 0, [(256, 2), (64, 4), (1, 64)])
                        gyb = _ap(gy_sb, k * 32 + 4 * q, [(16, 2), (1, 4), (0, 64)])
                        eo = _ap(e0, k * 512, [(256, 2), (64, 4), (1, 64)])
                        nc.vector.tensor_tensor(eo, zin, gyb, op=OP.mult)

                    # h4 tree + path sum on gp: (kp4, h4, b64) -> (k2, b64)
                    t1 = ep_pool.tile([128, 512], BF16, tag="t1")
                    i0 = _ap(e0, 0, [(256, 4), (1, 128)])
                    i1 = _ap(e0, 128, [(256, 4), (1, 128)])
                    to = _ap(t1, 0, [(128, 4), (1, 128)])
                    nc.gpsimd.tensor_tensor(to, i0, i1, op=OP.add)
                    t2 = ep_pool.tile([128, 256], BF16, tag="t2")
                    nc.gpsimd.tensor_tensor(
                        _ap(t2, 0, [(64, 4), (1, 64)]),
                        _ap(t1, 0, [(128, 4), (1, 64)]),
                        _ap(t1, 64, [(128, 4), (1, 64)]),
                        op=OP.add,
                    )
                    ctb = ep_pool.tile([128, 128], F32, tag="ctb")
                    nc.gpsimd.tensor_tensor(
                        _ap(ctb, 0, [(64, 2), (1, 64)]),
                        _ap(t2, 0, [(128, 2), (1, 64)]),
                        _ap(t2, 64, [(128, 2), (1, 64)]),
                        op=OP.add,
                    )
                    ctbs.append(ctb)

                    if q == 1:
                        acc01 = sg.tile([128, 128], F32)
                        nc.vector.tensor_add(acc01[:], ctbs[0][:], ctbs[1][:])
                    elif q == 3:
                        acc23 = sg.tile([128, 128], F32)
                        nc.vector.tensor_add(acc23[:], ctbs[2][:], ctbs[3][:])

            vout = sg.tile([128, 128], F32)
            nc.vector.tensor_add(vout[:], acc01[:], acc23[:])
            nc.scalar.dma_start(out=out_d, in_=vout[:])

    nc.compile()
    return nc


def _get_nc():
    if "nc" not in _CACHE:
        _CACHE["nc"] = _build_kernel()
    return _CACHE["nc"]


def pack_x(x: np.ndarray) -> np.ndarray:
    """[B,H,W,C] fp32 -> bf16 [core, hp, w, (c16, h2, b64)]."""
    import ml_dtypes

    xb = x.astype(ml_dtypes.bfloat16)
    xb = xb.transpose(2, 3, 1, 0)                 # [W, C, H, B]
    xb = xb.reshape(W, C, NCORES, NCH, 2, B)
    xb = xb.transpose(2, 3, 0, 1, 4, 5)            # [core, hp, W, C, h2, B]
    return np.ascontiguousarray(xb.reshape(NCORES, NCH, W, C * 2 * B))


def pack_host(inputs: dict):
    x = np.asarray(inputs["inputs"], dtype=np.float32)
    xp = pack_x(x)
    p = {n: np.asarray(inputs[n], dtype=np.float32).reshape(U)
         for n in ("a1", "a2", "s1", "s2", "ux", "uy")}
    prm = np.zeros((128, 12), dtype=np.float32)
    for i, n in enumerate(("a1", "a2", "s1", "s2", "uy", "ux")):
        prm[:, 2 * i] = p[n][:128]
        prm[:, 2 * i + 1] = p[n][128:]
    aux = np.zeros((NCORES, 1, 16), dtype=np.float32)
    for c in range(NCORES):
        aux[c, 0, :] = np.arange(c * HSH, (c + 1) * HSH, dtype=np.float32)
    return xp, aux, prm


def run(inputs: dict, trace: bool = False):
    """Run on 8 cores; returns (full_output, BassKernelResults)."""
    nc = _get_nc()
    xp, aux, prm = pack_host(inputs)
    in_maps = [
        {"x": xp[i], "aux": aux[i], "prm": prm} for i in range(NCORES)
    ]
    res = run_bass_kernel_spmd(
        nc, in_maps, core_ids=list(range(NCORES)), trace=trace
    )
    total = np.zeros((128, 2, 64), dtype=np.float64)
    for r in res.results:
        total += r["out"].astype(np.float64).reshape(128, 2, 64)
    out = total.transpose(2, 1, 0).reshape(B, U)
    out = out + np.asarray(inputs["bias"], dtype=np.float64).reshape(1, U)
    return out.astype(np.float32), res


def kernel(**inputs) -> np.ndarray:
    out, _ = run(inputs, trace=False)
    return out
